# revision 3
# baseline (speedup 1.0000x reference)
"""Cross-attention layer kernel for Trainium2, sharded over 8 NeuronCores.

Reference computation (B=2, N=2048 tokens, embed 1024, kv-dim 768, 16 heads x 64):
    Q = query @ Wq + bq;  K = key @ Wk + bk;  V = value @ Wv + bv
    att = softmax((Q K^T) * 16**-0.5);  out = (att V) @ Wo + bo

Sharding: 8 cores = (batch b in {0,1}) x (head-group g in {0..3}, 4 heads each).
Each core computes its head-group's Q/K/V projections (256-wide embed slice),
attention, and a partial output projection (Wo rows for its slice). Host sums
the 4 partials per batch and adds bo.

On-core layout is feature-major ("transposed"): activations are staged as
x^T (embed, tokens) so the contraction dim always sits on SBUF partitions.
Softmax runs max-free (logits are ~N(0, 0.7) here, exp cannot overflow):
E = exp(S^T * scale) per key-tile, O_unnorm^T = V_aug^T-style ones-augmented
matmul accumulates both numerator (64 rows) and denominator Z (row 65).
Matmuls run in float32r (reduced-precision fp32 PE mode, ~1e-4 rel err).
"""
import numpy as np

import concourse.bass as bass
import concourse.mybir as mybir
import concourse.tile as tile
from concourse import bacc
from concourse.bass_utils import run_bass_kernel_spmd

F32R = mybir.dt.float32r
F32 = mybir.dt.float32
EXP = mybir.ActivationFunctionType.Exp

P = 128          # SBUF partitions
N = 2048         # tokens (both query and kv sequence length)
CQ = 1024        # query embed dim
CKV = 768        # kv embed dim
D = 256          # per-core embed slice (4 heads x 64)
H = 4            # heads per core
DH = 64          # head dim
NT = N // P      # 16 token tiles
KQ = CQ // P     # 8 k-tiles for Q projection
KK = CKV // P    # 6 k-tiles for K/V projections
NIC = 1024       # attention i-chunk (query-token chunk)
SCALE = 16 ** -0.5


def build():
    nc = bacc.Bacc("TRN2", target_bir_lowering=False, debug=False)

    xq = nc.dram_tensor("xq", [CQ, N], F32R, kind="ExternalInput")
    xk = nc.dram_tensor("xk", [CKV, N], F32R, kind="ExternalInput")
    xv = nc.dram_tensor("xv", [CKV, N], F32R, kind="ExternalInput")
    wq = nc.dram_tensor("wq", [CQ, D], F32R, kind="ExternalInput")
    wk = nc.dram_tensor("wk", [CKV, D], F32R, kind="ExternalInput")
    wv = nc.dram_tensor("wv", [CKV, D], F32R, kind="ExternalInput")
    wo = nc.dram_tensor("wo", [D, CQ], F32R, kind="ExternalInput")
    bq = nc.dram_tensor("bq", [D], F32, kind="ExternalInput")
    bk = nc.dram_tensor("bk", [D], F32, kind="ExternalInput")
    bv = nc.dram_tensor("bv", [1, D], F32R, kind="ExternalInput")
    out = nc.dram_tensor("out", [N, CQ], F32, kind="ExternalOutput")

    with tile.TileContext(nc) as tc:
        with (
            tc.tile_pool(name="consts", bufs=1) as consts,
            tc.tile_pool(name="persist", bufs=1) as persist,
            tc.tile_pool(name="ps_a", bufs=2, space="PSUM") as ps_a,
            tc.tile_pool(name="ps_b", bufs=2, space="PSUM") as ps_b,
            tc.tile_pool(name="epool", bufs=3) as epool,
            tc.tile_pool(name="zpool", bufs=2) as zpool,
            tc.tile_pool(name="opool", bufs=3) as opool,
        ):
            # ---- constants ----
            wq_sb = consts.tile([P, KQ, D], F32R)
            wk_sb = consts.tile([P, KK, D], F32R)
            wv_sb = consts.tile([P, KK, D], F32R)
            wo_sb = consts.tile([P, 2, CQ], F32R)
            for k in range(KQ):
                nc.sync.dma_start(out=wq_sb[:, k, :], in_=wq[k * P:(k + 1) * P, :])
            for k in range(KK):
                nc.sync.dma_start(out=wk_sb[:, k, :], in_=wk[k * P:(k + 1) * P, :])
                nc.sync.dma_start(out=wv_sb[:, k, :], in_=wv[k * P:(k + 1) * P, :])
            for t in range(2):
                nc.sync.dma_start(out=wo_sb[:, t, :], in_=wo[t * P:(t + 1) * P, :])
            bq_sb = consts.tile([P, 2], F32)
            bk_sb = consts.tile([P, 2], F32)
            nc.sync.dma_start(out=bq_sb, in_=bq.rearrange("(t p) -> p t", p=P))
            nc.sync.dma_start(out=bk_sb, in_=bk.rearrange("(t p) -> p t", p=P))
            bv_sb = consts.tile([1, D], F32R)
            nc.sync.dma_start(out=bv_sb, in_=bv[:, :])
            ones1_f = consts.tile([1, P], F32)
            nc.vector.memset(ones1_f, 1.0)
            ones1 = consts.tile([1, P], F32R)
            nc.vector.tensor_copy(ones1, ones1_f)

            QT_sb = persist.tile([P, 2, N], F32R)   # Q^T: feature-major
            KT_sb = persist.tile([P, 2, N], F32R)
            V_sb = persist.tile([P, NT, H, DH + 1], F32R)  # V natural + ones col
            ON_sb = persist.tile([P, 2, N], F32R)   # normalized attn out, feature-major
            onesv_f = consts.tile([P, NT, H], F32)
            nc.vector.memset(onesv_f, 1.0)
            nc.vector.tensor_copy(V_sb[:, :, :, DH], onesv_f)

            # ---- K projection: KT[d, j] = sum_c Wk[c, d] xk^T[c, j] + bk ----
            with tc.tile_pool(name="xkp", bufs=1) as xkp:
                xk_sb = xkp.tile([P, KK, N], F32R)
                for k in range(KK):
                    nc.sync.dma_start(out=xk_sb[:, k, :], in_=xk[k * P:(k + 1) * P, :])
                for t in range(2):
                    for ic in range(2):
                        pk = ps_a.tile([P, NIC], F32, tag="A")
                        for k in range(KK):
                            for hf in range(2):
                                nc.tensor.matmul(
                                    pk[:, hf * 512:(hf + 1) * 512],
                                    wk_sb[:, k, t * P:(t + 1) * P],
                                    xk_sb[:, k, ic * NIC + hf * 512: ic * NIC + (hf + 1) * 512],
                                    start=(k == 0), stop=(k == KK - 1),
                                )
                        nc.vector.tensor_scalar_add(
                            KT_sb[:, t, ic * NIC:(ic + 1) * NIC], pk, bk_sb[:, t:t + 1]
                        )

            # ---- Q projection ----
            with tc.tile_pool(name="xqp", bufs=1) as xqp:
                xq_sb = xqp.tile([P, KQ, N], F32R)
                for k in range(KQ):
                    nc.sync.dma_start(out=xq_sb[:, k, :], in_=xq[k * P:(k + 1) * P, :])
                for t in range(2):
                    for ic in range(2):
                        pq = ps_a.tile([P, NIC], F32, tag="A")
                        for k in range(KQ):
                            for hf in range(2):
                                nc.tensor.matmul(
                                    pq[:, hf * 512:(hf + 1) * 512],
                                    wq_sb[:, k, t * P:(t + 1) * P],
                                    xq_sb[:, k, ic * NIC + hf * 512: ic * NIC + (hf + 1) * 512],
                                    start=(k == 0), stop=(k == KQ - 1),
                                )
                        nc.vector.tensor_scalar_add(
                            QT_sb[:, t, ic * NIC:(ic + 1) * NIC], pq, bq_sb[:, t:t + 1]
                        )

            # ---- V projection (natural layout): V[j, d] = xv^T-contraction + bv ----
            with tc.tile_pool(name="xvp", bufs=1) as xvp:
                xv_sb = xvp.tile([P, KK, N], F32R)
                for k in range(KK):
                    nc.sync.dma_start(out=xv_sb[:, k, :], in_=xv[k * P:(k + 1) * P, :])
                for jt in range(NT):
                    pv = ps_b.tile([P, D], F32, tag="B")
                    for k in range(KK):
                        nc.tensor.matmul(
                            pv,
                            xv_sb[:, k, jt * P:(jt + 1) * P],
                            wv_sb[:, k, :],
                            start=(k == 0), stop=False,
                        )
                    # bias via ones k-tile: V[j, d] += 1 * bv[d]
                    nc.tensor.matmul(pv, ones1, bv_sb, start=False, stop=True)
                    nc.vector.tensor_copy(
                        V_sb[:, jt, :, 0:DH],
                        pv.rearrange("p (h c) -> p h c", c=DH),
                    )

            # ---- attention per head ----
            for h in range(H):
                t = h // 2
                po = DH * (h % 2)
                for ic in range(2):
                    isl = slice(ic * NIC, (ic + 1) * NIC)
                    o_ps = ps_b.tile([DH + 1, NIC], F32, tag="B")
                    for j in range(NT):
                        s_ps = ps_a.tile([P, NIC], F32, tag="A")
                        for hf in range(2):
                            nc.tensor.matmul(
                                s_ps[:, hf * 512:(hf + 1) * 512],
                                KT_sb[po:po + DH, t, j * P:(j + 1) * P],
                                QT_sb[po:po + DH, t, ic * NIC + hf * 512: ic * NIC + (hf + 1) * 512],
                                start=True, stop=True,
                            )
                        e = epool.tile([P, NIC], F32R, tag="E")
                        nc.scalar.activation(e, s_ps, EXP, scale=SCALE)
                        for hf in range(2):
                            nc.tensor.matmul(
                                o_ps[:, hf * 512:(hf + 1) * 512],
                                V_sb[:, j, h, :],
                                e[:, hf * 512:(hf + 1) * 512],
                                start=(j == 0), stop=(j == NT - 1),
                            )
                    zinv = zpool.tile([1, NIC], F32, tag="zi")
                    nc.vector.reciprocal(zinv, o_ps[DH:DH + 1, :])
                    zbc = zpool.tile([DH, NIC], F32, tag="zb")
                    nc.gpsimd.partition_broadcast(zbc, zinv)
                    nc.vector.tensor_mul(ON_sb[po:po + DH, t, isl], o_ps[0:DH, :], zbc)

            # ---- output projection: out[i, e] = sum_f ON[f, i] wo[f, e] ----
            for it in range(NT):
                po2 = ps_a.tile([P, CQ], F32, tag="A")
                for ft in range(2):
                    for hf in range(2):
                        nc.tensor.matmul(
                            po2[:, hf * 512:(hf + 1) * 512],
                            ON_sb[:, ft, it * P:(it + 1) * P],
                            wo_sb[:, ft, hf * 512:(hf + 1) * 512],
                            start=(ft == 0), stop=(ft == 1),
                        )
                o_out = opool.tile([P, CQ], F32, tag="o")
                nc.vector.tensor_copy(o_out, po2)
                nc.sync.dma_start(out=out[it * P:(it + 1) * P, :], in_=o_out)

    nc.compile()
    return nc


def kernel(**inputs):
    query = np.asarray(inputs["query"], dtype=np.float32)
    key = np.asarray(inputs["key"], dtype=np.float32)
    value = np.asarray(inputs["value"], dtype=np.float32)
    Wq = np.asarray(inputs["Wq"], dtype=np.float32)
    bq = np.asarray(inputs["bq"], dtype=np.float32)
    Wk = np.asarray(inputs["Wk"], dtype=np.float32)
    bk = np.asarray(inputs["bk"], dtype=np.float32)
    Wv = np.asarray(inputs["Wv"], dtype=np.float32)
    bv = np.asarray(inputs["bv"], dtype=np.float32)
    Wo = np.asarray(inputs["Wo"], dtype=np.float32)
    bo = np.asarray(inputs["bo"], dtype=np.float32)

    B = query.shape[0]
    nc = build()

    xqT = [np.ascontiguousarray(query[b].T) for b in range(B)]
    xkT = [np.ascontiguousarray(key[b].T) for b in range(B)]
    xvT = [np.ascontiguousarray(value[b].T) for b in range(B)]

    in_maps = []
    for c in range(8):
        b, g = c // 4, c % 4
        sl = slice(g * D, (g + 1) * D)
        in_maps.append({
            "xq": xqT[b], "xk": xkT[b], "xv": xvT[b],
            "wq": np.ascontiguousarray(Wq[:, sl]),
            "wk": np.ascontiguousarray(Wk[:, sl]),
            "wv": np.ascontiguousarray(Wv[:, sl]),
            "wo": np.ascontiguousarray(Wo[sl, :]),
            "bq": np.ascontiguousarray(bq[sl]),
            "bk": np.ascontiguousarray(bk[sl]),
            "bv": np.ascontiguousarray(bv[sl]).reshape(1, D),
        })

    res = run_bass_kernel_spmd(nc, in_maps, core_ids=list(range(8)))
    parts = [r["out"] for r in res.results]

    final = np.empty((B, N, CQ), dtype=np.float32)
    for b in range(B):
        acc = np.zeros((N, CQ), dtype=np.float64)
        for g in range(4):
            acc += parts[4 * b + g]
        acc += bo
        final[b] = acc.astype(np.float32)
    return final
